# revision 22
# baseline (speedup 1.0000x reference)
"""Trainium2 Bass kernel for nn_Actor (diagonal complex LRU, last-step output).

Math: the reference runs an associative scan x_t = lam*x_{t-1} + (gamma*B) u_t
over L=2048 steps and keeps only y[:, -1, :].  The last state collapses to
    x_L[n] = sum_t lam[n]^(L-1-t) * (Bmat @ u_t)[n]
Since |lam| <= 0.99 the sum is truncated: modes are sorted by |lam| (a free
host-side permutation of the diagonal) so the slow half uses the last K=256
steps and the fast half (|lam| <~ 0.76) only the last 128.

Per core (8 batches), on device:
    v[n, b, h] = sum_t W[t, n] * u[b, t, h]      (TensorE, contracts time)
    x[n, b]    = sum_h Bmat[n, h] * v[n, b, h]   (VectorE stt + reduce)
    y[b, o]    = Re(C x)[b, o] + (D u_last)[b, o] (TensorE, tiny)
W[t, n] = lam[n]^(K-1-t), gamma-folded B, transposed C/D are tiny
parameter-only tables computed host-side and shipped as packed bf16
constants (3 input DMAs total); u's tail is pre-transposed/cast host-side.

Sharding: data-parallel over batch (64 -> 8 per core) on 8 NeuronCores,
no collectives; host concatenates per-core outputs.
"""

import sys

sys.path.insert(0, "/opt/trn_rl_repo")

import ml_dtypes
import numpy as np

import concourse.bass as bass
import concourse.tile as tile
from concourse import bacc, mybir
from concourse.bass_utils import run_bass_kernel_spmd

B, L, H, O, N = 64, 2048, 128, 128, 256
NCORES = 8
BS = B // NCORES  # 8 batches per core
K = 256  # truncated window (last K steps) for the slow half
KT = K // 128  # 2 time tiles of 128
F32 = mybir.dt.float32
BF16 = mybir.dt.bfloat16
BF = ml_dtypes.bfloat16

# packed const layout: offsets into cst [128, CSTW]
OFF_BRE = 0  # [128, 2, 128] (nh, h)
OFF_BIM = 256  # [128, 2, 128]
OFF_CRET = 512  # [128, 2, 128] (nh, o)
OFF_NCIMT = 768  # [128, 2, 128]
OFF_DT = 1024  # [128, 128]
OFF_ULT = 1152  # [128, 8]
CSTW = 1160
# wcat lanes: (ri, j) lhsT blocks [128, 128] each
# 0: re j0 nh0, 1: im j0 nh0, 2: re j1 nh0, 3: im j1 nh0, 4: re j1 nh1, 5: im j1 nh1
WLANES = 6


def build():
    nc = bacc.Bacc("TRN2", target_bir_lowering=False, debug=False)

    # p1: nh1's W (K=64 window, contraction=64) + last-64-step u slab.
    p1_d = nc.dram_tensor("p1", [64, 2 * 128 + BS * H], BF16, kind="ExternalInput")
    # p2: nh0's W lanes 0-3 + both full u tiles (j1 then j0).
    p2_d = nc.dram_tensor("p2", [128, 4 * 128 + 2 * BS * H], BF16, kind="ExternalInput")
    cst_d = nc.dram_tensor("cst", [128, CSTW], BF16, kind="ExternalInput")
    out_d = nc.dram_tensor("out", [O, BS], F32, kind="ExternalOutput")

    mult = mybir.AluOpType.mult
    add = mybir.AluOpType.add

    with tile.TileContext(nc) as tc:
        with (
            tc.tile_pool(name="const", bufs=1) as cp,
            tc.tile_pool(name="wk", bufs=1) as wk,
            tc.tile_pool(name="psum", bufs=1, space=bass.MemorySpace.PSUM) as pp,
        ):
            # ---- 4 input DMAs total ------------------------------------
            # Triggers spread across engine queues (each DIRECT2D trigger
            # occupies its issuing queue ~0.6us; sync alone would serialize).
            # Single trigger queue, strict priority order: bytes ahead of a
            # tensor in the DMA queues are what delay it.  Phase-1 tensors
            # (nh1's W + u tail tile) ride one packed DMA.
            p1 = cp.tile([64, 2 * 128 + BS * H], BF16, tag="p1")
            nc.sync.dma_start(p1[:], p1_d[:, :])
            p2a = cp.tile([128, 4 * 128 + BS * H], BF16, tag="p2a")
            nc.sync.dma_start(p2a[:], p2_d[:, 0 : 512 + BS * H])
            cst = cp.tile([128, CSTW], BF16, tag="cst")
            nc.sync.dma_start(cst[:], cst_d[:, :])
            p2b = cp.tile([128, BS * H], BF16, tag="p2b")
            nc.sync.dma_start(p2b[:], p2_d[:, 512 + BS * H :])


            # ---- PSUM accumulators: one [b, h] tile per (nh, ri) -------
            pvs = [
                [
                    pp.tile([128, BS, H], F32, tag=f"pv{nh}{ri}", name=f"pv{nh}{ri}")
                    for ri in range(2)
                ]
                for nh in range(2)
            ]
            ds = [[None, None], [None, None]]  # ds[nh] = [d1, d2] in SBUF bf16

            def mm_nh1():
                # p1 = [w-lanes 4,5 (64-row) | u-last64]: single j, per-ri bank.
                for ri in range(2):
                    lhsT = p1[:, ri * 128 : (ri + 1) * 128]
                    for half in range(2):
                        nc.tensor.matmul(
                            pvs[1][ri][:, half * 4 : (half + 1) * 4, :],
                            lhsT,
                            p1[:, 256 + half * 512 : 256 + (half + 1) * 512],
                            start=True,
                            stop=True,
                        )

            def mm_nh0():
                # Bank-major: finish ri0's full j1+j0 accumulation first so its
                # PSUM->SBUF copy (and the vector chain) starts earliest.
                # j1 (u in p2a) first since p2b (u-j0) lands last.
                for ri in range(2):
                    for j, ub, uoff in ((1, p2a, 512), (0, p2b, 0)):
                        lhsT = p2a[:, (2 * j + ri) * 128 : (2 * j + ri + 1) * 128]
                        for half in range(2):
                            nc.tensor.matmul(
                                pvs[0][ri][:, half * 4 : (half + 1) * 4, :],
                                lhsT,
                                ub[:, uoff + half * 512 : uoff + (half + 1) * 512],
                                start=j == 1,
                                stop=j == 0,
                            )

            def epilogue(nh):
                bre_b = cst[:, OFF_BRE + nh * 128 : OFF_BRE + (nh + 1) * 128][
                    :, None, :
                ].broadcast_to([128, BS, H])
                bim_b = cst[:, OFF_BIM + nh * 128 : OFF_BIM + (nh + 1) * 128][
                    :, None, :
                ].broadcast_to([128, BS, H])
                sub = mybir.AluOpType.subtract
                sv0 = wk.tile([128, BS, H], BF16, tag="sv0", name=f"sv0_{nh}")
                nc.scalar.copy(sv0[:], pvs[nh][0][:])
                sv1 = wk.tile([128, BS, H], BF16, tag="sv1", name=f"sv1_{nh}")
                nc.scalar.copy(sv1[:], pvs[nh][1][:])
                t1 = wk.tile([128, BS, H], BF16, tag="t1", name=f"t1_{nh}")
                nc.vector.tensor_tensor(t1[:], sv0[:], bre_b, mult)
                t4 = wk.tile([128, BS, H], BF16, tag="t4", name=f"t4_{nh}")
                nc.vector.tensor_tensor(t4[:], sv0[:], bim_b, mult)
                t2 = wk.tile([128, BS, H], BF16, tag="t2", name=f"t2_{nh}")
                nc.vector.tensor_tensor(t2[:], sv1[:], bim_b, mult)
                d1 = wk.tile([128, BS, H], BF16, tag=f"d1_{nh}", name=f"d1_{nh}")
                nc.vector.tensor_tensor(d1[:], t1[:], t2[:], sub)
                t3 = wk.tile([128, BS, H], BF16, tag="t3", name=f"t3_{nh}")
                nc.vector.tensor_tensor(t3[:], sv1[:], bre_b, mult)
                d2 = wk.tile([128, BS, H], BF16, tag=f"d2_{nh}", name=f"d2_{nh}")
                nc.vector.tensor_tensor(d2[:], t3[:], t4[:], add)
                ds[nh][0] = d1
                ds[nh][1] = d2

            # nh1 (K=64 fast modes) first: its packet lands first and its
            # epilogue overlaps nh0's matmuls.
            mm_nh1()
            epilogue(1)
            mm_nh0()
            epilogue(0)

            # Project d1/d2 through C on the PE (contraction over n), all
            # four (nh, comp) combos accumulating into one PSUM py_oh[o,b,h];
            # a single h-reduce then replaces four vector reduces.
            creT = [cst[:, OFF_CRET + i * 128 : OFF_CRET + (i + 1) * 128] for i in range(2)]
            ncimT = [
                cst[:, OFF_NCIMT + i * 128 : OFF_NCIMT + (i + 1) * 128] for i in range(2)
            ]
            py_oh = pp.tile([128, BS, H], F32, tag="pv10", name="py_oh")
            for gi, nh in enumerate((1, 0)):
                for ci, cT in ((0, creT[nh]), (1, ncimT[nh])):
                    for half in range(2):
                        nc.tensor.matmul(
                            py_oh[:, half * 4 : (half + 1) * 4, :],
                            cT,
                            ds[nh][ci][:, half * 4 : (half + 1) * 4, :],
                            start=gi == 0 and ci == 0,
                            stop=False,
                        )
            # D @ u_last lands in the h=0 slice of each bank and closes the
            # accumulation; the h-reduce then yields y in one pass.
            for half in range(2):
                nc.tensor.matmul(
                    py_oh[:, half * 4 : (half + 1) * 4, 0:1],
                    cst[:, OFF_DT : OFF_DT + 128],
                    cst[:, OFF_ULT + half * 4 : OFF_ULT + (half + 1) * 4],
                    start=False,
                    stop=True,
                )
            y_sb = cp.tile([O, BS], F32, tag="y_sb")
            for half in range(2):
                nc.vector.tensor_reduce(
                    y_sb[:, half * 4 : (half + 1) * 4],
                    py_oh[:, half * 4 : (half + 1) * 4, :],
                    mybir.AxisListType.X,
                    add,
                )
            nc.sync.dma_start(out_d[:, :], y_sb[:])

    nc.compile()
    return nc


_NC_CACHE = None


def _get_nc():
    global _NC_CACHE
    if _NC_CACHE is None:
        _NC_CACHE = build()
    return _NC_CACHE


def _make_in_maps(inputs):
    u = np.asarray(inputs["dynamics_disturbance_time_window"], np.float32)
    nu = np.asarray(inputs["nu_log"], np.float64)
    th = np.asarray(inputs["theta_log"], np.float64)
    gm = np.asarray(inputs["gamma_log"], np.float64)

    lam = np.exp(-np.exp(nu) + 1j * np.exp(th))  # [N] complex128
    perm = np.argsort(-np.abs(lam), kind="stable")  # slow modes first
    lam_s = lam[perm]
    expo = np.arange(K - 1, -1, -1, dtype=np.float64)
    W = lam_s[None, :] ** expo[:, None]  # [K, N] (sorted mode order)
    Wre = W.real.astype(np.float32).astype(BF)
    Wim = W.imag.astype(np.float32).astype(BF)
    w03 = np.empty((128, 4, 128), BF)
    w03[:, 0] = Wre[0:128, 0:128]
    w03[:, 1] = Wim[0:128, 0:128]
    w03[:, 2] = Wre[128:256, 0:128]
    w03[:, 3] = Wim[128:256, 0:128]
    w45 = np.empty((64, 2, 128), BF)
    w45[:, 0] = Wre[192:256, 128:256]
    w45[:, 1] = Wim[192:256, 128:256]

    g = np.exp(gm)[:, None]
    bre = (np.asarray(inputs["B_re"], np.float64) * g)[perm]
    bim = (np.asarray(inputs["B_im"], np.float64) * g)[perm]
    cre = np.asarray(inputs["C_re"], np.float64)[:, perm]
    cim = np.asarray(inputs["C_im"], np.float64)[:, perm]
    dT = np.asarray(inputs["D"], np.float32).T

    cst = np.zeros((128, CSTW), np.float32)
    cst[:, OFF_BRE : OFF_BRE + 256] = bre.reshape(2, 128, H).transpose(1, 0, 2).reshape(128, 256)
    cst[:, OFF_BIM : OFF_BIM + 256] = bim.reshape(2, 128, H).transpose(1, 0, 2).reshape(128, 256)
    creT = cre.T.reshape(2, 128, O)  # [nh, n', o]
    ncimT = (-cim).T.reshape(2, 128, O)
    cst[:, OFF_CRET : OFF_CRET + 256] = creT.transpose(1, 0, 2).reshape(128, 256)
    cst[:, OFF_NCIMT : OFF_NCIMT + 256] = ncimT.transpose(1, 0, 2).reshape(128, 256)
    cst[:, OFF_DT : OFF_DT + 128] = dT

    tail = u[:, L - K :, :].transpose(1, 0, 2).astype(BF)  # [K, B, H]
    ul = u[:, L - 1, :].T  # [H, B] f32
    in_maps = []
    for i in range(NCORES):
        sl = slice(i * BS, (i + 1) * BS)
        ut = np.ascontiguousarray(tail[:, sl, :]).reshape(KT, 128, BS, H)
        ci = cst.copy()
        ci[:, OFF_ULT : OFF_ULT + BS] = ul[:, sl]
        in_maps.append(
            {
                "p1": np.concatenate(
                    [w45.reshape(64, 256), ut[1, 64:].reshape(64, BS * H)], axis=1
                ),
                "p2": np.concatenate(
                    [
                        w03.reshape(128, 512),
                        ut[1].reshape(128, BS * H),
                        ut[0].reshape(128, BS * H),
                    ],
                    axis=1,
                ),
                "cst": ci.astype(BF),
            }
        )
    return in_maps


def _ensure_profile_hook():
    """The agent image's antenv lacks axon_hooks; shim it and register the
    ctypes NTFF hook so run_bass_kernel_spmd(trace=True) can profile."""
    import types

    if "antenv.axon_hooks" in sys.modules:
        return
    mod = types.ModuleType("antenv.axon_hooks")
    mod._hook = None
    mod.set_axon_ntff_profile_hook = lambda h: setattr(mod, "_hook", h)
    mod.get_axon_ntff_profile_hook = lambda: mod._hook
    sys.modules["antenv.axon_hooks"] = mod
    try:
        from trn_agent_boot.trn_boot import _ntff_profile_via_ctypes

        mod._hook = _ntff_profile_via_ctypes("/opt/axon/libaxon_pjrt.so")
    except Exception as e:
        print(f"profile hook setup failed: {e}", file=sys.stderr)


def run(inputs, trace=False, tmpdir=None):
    if trace:
        _ensure_profile_hook()
    nc = _get_nc()
    in_maps = _make_in_maps(inputs)
    res = run_bass_kernel_spmd(
        nc, in_maps, list(range(NCORES)), trace=trace, tmpdir=tmpdir
    )
    out = np.concatenate([res.results[i]["out"].T for i in range(NCORES)], axis=0)
    return out.astype(np.float32), res


def kernel(**inputs):
    out, _ = run(inputs, trace=False)
    return out


# revision 23
# speedup vs baseline: 1.0127x; 1.0127x over previous
"""Trainium2 Bass kernel for nn_Actor (diagonal complex LRU, last-step output).

Math: the reference runs an associative scan x_t = lam*x_{t-1} + (gamma*B) u_t
over L=2048 steps and keeps only y[:, -1, :].  The last state collapses to
    x_L[n] = sum_t lam[n]^(L-1-t) * (Bmat @ u_t)[n]
Since |lam| <= 0.99 the sum is truncated: modes are sorted by |lam| (a free
host-side permutation of the diagonal) so the slow half uses the last K=256
steps and the fast half (|lam| <~ 0.76) only the last 128.

Per core (8 batches), on device:
    v[n, b, h] = sum_t W[t, n] * u[b, t, h]      (TensorE, contracts time)
    x[n, b]    = sum_h Bmat[n, h] * v[n, b, h]   (VectorE stt + reduce)
    y[b, o]    = Re(C x)[b, o] + (D u_last)[b, o] (TensorE, tiny)
W[t, n] = lam[n]^(K-1-t), gamma-folded B, transposed C/D are tiny
parameter-only tables computed host-side and shipped as packed bf16
constants (3 input DMAs total); u's tail is pre-transposed/cast host-side.

Sharding: data-parallel over batch (64 -> 8 per core) on 8 NeuronCores,
no collectives; host concatenates per-core outputs.
"""

import sys

sys.path.insert(0, "/opt/trn_rl_repo")

import ml_dtypes
import numpy as np

import concourse.bass as bass
import concourse.tile as tile
from concourse import bacc, mybir
from concourse.bass_utils import run_bass_kernel_spmd

B, L, H, O, N = 64, 2048, 128, 128, 256
NCORES = 8
BS = B // NCORES  # 8 batches per core
K = 256  # truncated window (last K steps) for the slow half
KT = K // 128  # 2 time tiles of 128
F32 = mybir.dt.float32
BF16 = mybir.dt.bfloat16
BF = ml_dtypes.bfloat16

# packed const layout: offsets into cst [128, CSTW]
OFF_BRE = 0  # [128, 2, 128] (nh, h)
OFF_BIM = 256  # [128, 2, 128]
OFF_CRET = 512  # [128, 2, 128] (nh, o)
OFF_NCIMT = 768  # [128, 2, 128]
OFF_DT = 1024  # [128, 128]
OFF_ULT = 1152  # [128, 8]
CSTW = 1160
# wcat lanes: (ri, j) lhsT blocks [128, 128] each
# 0: re j0 nh0, 1: im j0 nh0, 2: re j1 nh0, 3: im j1 nh0, 4: re j1 nh1, 5: im j1 nh1
WLANES = 6


def build():
    nc = bacc.Bacc("TRN2", target_bir_lowering=False, debug=False)

    # p1: nh1's W (K=64 window, contraction=64) + last-64-step u slab.
    p1_d = nc.dram_tensor("p1", [64, 2 * 128 + BS * H], BF16, kind="ExternalInput")
    # p2: nh0's W lanes 0-3 + both full u tiles (j1 then j0).
    p2_d = nc.dram_tensor("p2", [128, 4 * 128 + 2 * BS * H], BF16, kind="ExternalInput")
    cst_d = nc.dram_tensor("cst", [128, CSTW], BF16, kind="ExternalInput")
    out_d = nc.dram_tensor("out", [O, BS], F32, kind="ExternalOutput")

    mult = mybir.AluOpType.mult
    add = mybir.AluOpType.add

    with tile.TileContext(nc) as tc:
        with (
            tc.tile_pool(name="const", bufs=1) as cp,
            tc.tile_pool(name="wk", bufs=1) as wk,
            tc.tile_pool(name="psum", bufs=1, space=bass.MemorySpace.PSUM) as pp,
        ):
            # ---- 4 input DMAs total ------------------------------------
            # Triggers spread across engine queues (each DIRECT2D trigger
            # occupies its issuing queue ~0.6us; sync alone would serialize).
            # Single trigger queue, strict priority order: bytes ahead of a
            # tensor in the DMA queues are what delay it.  Phase-1 tensors
            # (nh1's W + u tail tile) ride one packed DMA.
            p1 = cp.tile([64, 2 * 128 + BS * H], BF16, tag="p1")
            nc.sync.dma_start(p1[:], p1_d[:, :])
            p2a = cp.tile([128, 4 * 128 + BS * H], BF16, tag="p2a")
            nc.sync.dma_start(p2a[:], p2_d[:, 0 : 512 + BS * H])
            cst = cp.tile([128, CSTW], BF16, tag="cst")
            nc.sync.dma_start(cst[:], cst_d[:, :])
            p2b = cp.tile([128, BS * H], BF16, tag="p2b")
            nc.sync.dma_start(p2b[:], p2_d[:, 512 + BS * H :])


            # ---- PSUM accumulators: one [b, h] tile per (nh, ri) -------
            pvs = [
                [
                    pp.tile([128, BS, H], F32, tag=f"pv{nh}{ri}", name=f"pv{nh}{ri}")
                    for ri in range(2)
                ]
                for nh in range(2)
            ]
            ds = [[None, None], [None, None]]  # ds[nh] = [d1, d2] in SBUF bf16

            def mm_nh1():
                # p1 = [w-lanes 4,5 (64-row) | u-last64]: single j, per-ri bank.
                for ri in range(2):
                    lhsT = p1[:, ri * 128 : (ri + 1) * 128]
                    for half in range(2):
                        nc.tensor.matmul(
                            pvs[1][ri][:, half * 4 : (half + 1) * 4, :],
                            lhsT,
                            p1[:, 256 + half * 512 : 256 + (half + 1) * 512],
                            start=True,
                            stop=True,
                        )

            def mm_nh0():
                # Bank-major: finish ri0's full j1+j0 accumulation first so its
                # PSUM->SBUF copy (and the vector chain) starts earliest.
                # j1 (u in p2a) first since p2b (u-j0) lands last.
                for ri in range(2):
                    for j, ub, uoff in ((1, p2a, 512), (0, p2b, 0)):
                        lhsT = p2a[:, (2 * j + ri) * 128 : (2 * j + ri + 1) * 128]
                        for half in range(2):
                            nc.tensor.matmul(
                                pvs[0][ri][:, half * 4 : (half + 1) * 4, :],
                                lhsT,
                                ub[:, uoff + half * 512 : uoff + (half + 1) * 512],
                                start=j == 1,
                                stop=j == 0,
                            )

            def epilogue(nh):
                bre_b = cst[:, OFF_BRE + nh * 128 : OFF_BRE + (nh + 1) * 128][
                    :, None, :
                ].broadcast_to([128, BS, H])
                bim_b = cst[:, OFF_BIM + nh * 128 : OFF_BIM + (nh + 1) * 128][
                    :, None, :
                ].broadcast_to([128, BS, H])
                sub = mybir.AluOpType.subtract
                sv0 = wk.tile([128, BS, H], BF16, tag="sv0", name=f"sv0_{nh}")
                nc.scalar.copy(sv0[:], pvs[nh][0][:])
                sv1 = wk.tile([128, BS, H], BF16, tag="sv1", name=f"sv1_{nh}")
                nc.scalar.copy(sv1[:], pvs[nh][1][:])
                t1 = wk.tile([128, BS, H], BF16, tag="t1", name=f"t1_{nh}")
                nc.vector.tensor_tensor(t1[:], sv0[:], bre_b, mult)
                t4 = wk.tile([128, BS, H], BF16, tag="t4", name=f"t4_{nh}")
                nc.vector.tensor_tensor(t4[:], sv0[:], bim_b, mult)
                t2 = wk.tile([128, BS, H], BF16, tag="t2", name=f"t2_{nh}")
                nc.vector.tensor_tensor(t2[:], sv1[:], bim_b, mult)
                d1 = wk.tile([128, BS, H], BF16, tag=f"d1_{nh}", name=f"d1_{nh}")
                nc.vector.tensor_tensor(d1[:], t1[:], t2[:], sub)
                t3 = wk.tile([128, BS, H], BF16, tag="t3", name=f"t3_{nh}")
                nc.vector.tensor_tensor(t3[:], sv1[:], bre_b, mult)
                d2 = wk.tile([128, BS, H], BF16, tag=f"d2_{nh}", name=f"d2_{nh}")
                nc.vector.tensor_tensor(d2[:], t3[:], t4[:], add)
                ds[nh][0] = d1
                ds[nh][1] = d2

            # nh1 (K=64 fast modes) first: its packet lands first and its
            # epilogue overlaps nh0's matmuls.
            mm_nh1()
            epilogue(1)
            mm_nh0()
            epilogue(0)

            # Project d1/d2 through C on the PE (contraction over n), all
            # four (nh, comp) combos accumulating into one PSUM py_oh[o,b,h];
            # a single h-reduce then replaces four vector reduces.
            creT = [cst[:, OFF_CRET + i * 128 : OFF_CRET + (i + 1) * 128] for i in range(2)]
            ncimT = [
                cst[:, OFF_NCIMT + i * 128 : OFF_NCIMT + (i + 1) * 128] for i in range(2)
            ]
            py_oh = pp.tile([128, BS, H], F32, tag="pv10", name="py_oh")
            yd = pp.tile([O, BS], F32, tag="pv11", name="yd")
            for gi, nh in enumerate((1, 0)):
                for ci, cT in ((0, creT[nh]), (1, ncimT[nh])):
                    for half in range(2):
                        nc.tensor.matmul(
                            py_oh[:, half * 4 : (half + 1) * 4, :],
                            cT,
                            ds[nh][ci][:, half * 4 : (half + 1) * 4, :],
                            start=gi == 0 and ci == 0,
                            stop=gi == 1 and ci == 1,
                        )
            nc.tensor.matmul(
                yd[:],
                cst[:, OFF_DT : OFF_DT + 128],
                cst[:, OFF_ULT : OFF_ULT + BS],
                start=True,
                stop=True,
            )
            ytmp = wk.tile([O, BS], F32, tag="ytmp")
            for half in range(2):
                nc.vector.tensor_reduce(
                    ytmp[:, half * 4 : (half + 1) * 4],
                    py_oh[:, half * 4 : (half + 1) * 4, :],
                    mybir.AxisListType.X,
                    add,
                )
            y_sb = cp.tile([O, BS], F32, tag="y_sb")
            nc.vector.tensor_tensor(y_sb[:], ytmp[:], yd[:], add)
            nc.sync.dma_start(out_d[:, :], y_sb[:])

    nc.compile()
    return nc


_NC_CACHE = None


def _get_nc():
    global _NC_CACHE
    if _NC_CACHE is None:
        _NC_CACHE = build()
    return _NC_CACHE


def _make_in_maps(inputs):
    u = np.asarray(inputs["dynamics_disturbance_time_window"], np.float32)
    nu = np.asarray(inputs["nu_log"], np.float64)
    th = np.asarray(inputs["theta_log"], np.float64)
    gm = np.asarray(inputs["gamma_log"], np.float64)

    lam = np.exp(-np.exp(nu) + 1j * np.exp(th))  # [N] complex128
    perm = np.argsort(-np.abs(lam), kind="stable")  # slow modes first
    lam_s = lam[perm]
    expo = np.arange(K - 1, -1, -1, dtype=np.float64)
    W = lam_s[None, :] ** expo[:, None]  # [K, N] (sorted mode order)
    Wre = W.real.astype(np.float32).astype(BF)
    Wim = W.imag.astype(np.float32).astype(BF)
    w03 = np.empty((128, 4, 128), BF)
    w03[:, 0] = Wre[0:128, 0:128]
    w03[:, 1] = Wim[0:128, 0:128]
    w03[:, 2] = Wre[128:256, 0:128]
    w03[:, 3] = Wim[128:256, 0:128]
    w45 = np.empty((64, 2, 128), BF)
    w45[:, 0] = Wre[192:256, 128:256]
    w45[:, 1] = Wim[192:256, 128:256]

    g = np.exp(gm)[:, None]
    bre = (np.asarray(inputs["B_re"], np.float64) * g)[perm]
    bim = (np.asarray(inputs["B_im"], np.float64) * g)[perm]
    cre = np.asarray(inputs["C_re"], np.float64)[:, perm]
    cim = np.asarray(inputs["C_im"], np.float64)[:, perm]
    dT = np.asarray(inputs["D"], np.float32).T

    cst = np.zeros((128, CSTW), np.float32)
    cst[:, OFF_BRE : OFF_BRE + 256] = bre.reshape(2, 128, H).transpose(1, 0, 2).reshape(128, 256)
    cst[:, OFF_BIM : OFF_BIM + 256] = bim.reshape(2, 128, H).transpose(1, 0, 2).reshape(128, 256)
    creT = cre.T.reshape(2, 128, O)  # [nh, n', o]
    ncimT = (-cim).T.reshape(2, 128, O)
    cst[:, OFF_CRET : OFF_CRET + 256] = creT.transpose(1, 0, 2).reshape(128, 256)
    cst[:, OFF_NCIMT : OFF_NCIMT + 256] = ncimT.transpose(1, 0, 2).reshape(128, 256)
    cst[:, OFF_DT : OFF_DT + 128] = dT

    tail = u[:, L - K :, :].transpose(1, 0, 2).astype(BF)  # [K, B, H]
    ul = u[:, L - 1, :].T  # [H, B] f32
    in_maps = []
    for i in range(NCORES):
        sl = slice(i * BS, (i + 1) * BS)
        ut = np.ascontiguousarray(tail[:, sl, :]).reshape(KT, 128, BS, H)
        ci = cst.copy()
        ci[:, OFF_ULT : OFF_ULT + BS] = ul[:, sl]
        in_maps.append(
            {
                "p1": np.concatenate(
                    [w45.reshape(64, 256), ut[1, 64:].reshape(64, BS * H)], axis=1
                ),
                "p2": np.concatenate(
                    [
                        w03.reshape(128, 512),
                        ut[1].reshape(128, BS * H),
                        ut[0].reshape(128, BS * H),
                    ],
                    axis=1,
                ),
                "cst": ci.astype(BF),
            }
        )
    return in_maps


def _ensure_profile_hook():
    """The agent image's antenv lacks axon_hooks; shim it and register the
    ctypes NTFF hook so run_bass_kernel_spmd(trace=True) can profile."""
    import types

    if "antenv.axon_hooks" in sys.modules:
        return
    mod = types.ModuleType("antenv.axon_hooks")
    mod._hook = None
    mod.set_axon_ntff_profile_hook = lambda h: setattr(mod, "_hook", h)
    mod.get_axon_ntff_profile_hook = lambda: mod._hook
    sys.modules["antenv.axon_hooks"] = mod
    try:
        from trn_agent_boot.trn_boot import _ntff_profile_via_ctypes

        mod._hook = _ntff_profile_via_ctypes("/opt/axon/libaxon_pjrt.so")
    except Exception as e:
        print(f"profile hook setup failed: {e}", file=sys.stderr)


def run(inputs, trace=False, tmpdir=None):
    if trace:
        _ensure_profile_hook()
    nc = _get_nc()
    in_maps = _make_in_maps(inputs)
    res = run_bass_kernel_spmd(
        nc, in_maps, list(range(NCORES)), trace=trace, tmpdir=tmpdir
    )
    out = np.concatenate([res.results[i]["out"].T for i in range(NCORES)], axis=0)
    return out.astype(np.float32), res


def kernel(**inputs):
    out, _ = run(inputs, trace=False)
    return out
